# revision 14
# baseline (speedup 1.0000x reference)
"""GQA attention layer (B=1, S=2048, D=4096, H=32, KV=8, HD=128) on 8 TRN2
NeuronCores. Tensor-parallel over the 8 KV-head groups; NO collective: each
core computes a partial y = O_local @ wo_rows_local over the full output dim
and the host sums the 8 partials during unshard.

All matmuls in bf16 (fp32 PSUM accumulation). Phase 1 is weight-stationary,
chunk-major (6 live PSUM accumulators, one per m-tile) so compute tracks the
x-chunk DMA arrival during the startup quarter; qT/kT are produced directly
(RoPE applied in transposed layout at quarter-end eviction). The sync DMA
ring carries ident -> wt-m0/x-q0 interleaved -> wt-m1..5 -> x-q1.. in exact
FIFO order so every tensor lands just-in-time; warm-up matmuls on ident flip
the PE HAM clock gate to 8/8 before real work arrives.

Attention runs transposed (scores^T = kT-tiles as lhsT against qT); score
tiles are processed in PAIRS sharing one wide exp ACTIVATE. Softmax sums run
as M=1 matmuls col-tiled 4-way into separate PE column groups (concurrent on
the array, ~4x cheaper than serial row-sum matmuls); a zero-weight N=1
matmul pre-clears the PSUM bank's has_written bits so the col groups can all
accumulate with start=False. The 1/l broadcast matmul is bf16.

Phase 2 is group-outer/head-inner and each group's phase-3 out-projection
chunk (32 dt x 4 head matmuls into a [128,512] PSUM tile) is emitted right
after the group finishes, so the Tile scheduler hoists phase-3 matmuls into
phase-2's exp-wait stalls. wo streams into the SBUF slot freed by the x
double-buffer (same tag) mid-phase-1. Output is written per (dt, group)
chunk, alternating sync/gpsimd DMA queues.
"""
import numpy as np
import ml_dtypes

S = 2048
D = 4096
HD = 128
QH = 4            # q heads per core
NCORES = 8
ST = S // 128     # 16 s-tiles
DK = D // 128     # 32 contraction tiles
G = 4             # q groups per head
SG = S // G       # 512 q positions per group
NM = 6            # phase-1 m-tiles per core: v, k, q0..q3
SCALE = 1.0 / np.sqrt(128.0)

_CACHE = {}


def _build():
    import concourse.mybir as mybir
    import concourse.tile as tile
    from concourse import bacc

    f32 = mybir.dt.float32
    bf16 = mybir.dt.bfloat16
    mul = mybir.AluOpType.mult
    add = mybir.AluOpType.add
    nc = bacc.Bacc("TRN2", target_bir_lowering=False, debug=False,
                   num_devices=NCORES)

    xtd = nc.dram_tensor("xtd", [128, 4, DK, SG], bf16,
                         kind="ExternalInput").ap()
    wt = nc.dram_tensor("wt", [128, DK, NM, 128], bf16,
                        kind="ExternalInput").ap()
    wo = nc.dram_tensor("wo", [128, QH, DK, 128], bf16,
                        kind="ExternalInput").ap()
    cs = nc.dram_tensor("cs", [128, S], f32, kind="ExternalInput").ap()
    tri = nc.dram_tensor("tri", [128, 128], bf16, kind="ExternalInput").ap()
    tri2 = nc.dram_tensor("tri2", [128, 256], bf16, kind="ExternalInput").ap()
    onesc = nc.dram_tensor("onesc", [128, 1], bf16, kind="ExternalInput").ap()
    onesr = nc.dram_tensor("onesr", [1, 128], bf16, kind="ExternalInput").ap()
    sel4 = nc.dram_tensor("sel4", [128, 1], bf16, kind="ExternalInput").ap()
    ident = nc.dram_tensor("ident", [128, 128], bf16,
                           kind="ExternalInput").ap()
    yt = nc.dram_tensor("yt", [D, S], bf16, kind="ExternalOutput").ap()

    with tile.TileContext(nc) as tc:
        with (
            tc.tile_pool(name="const", bufs=1) as constp,
            tc.tile_pool(name="resid", bufs=1) as resid,
            tc.tile_pool(name="xq_pool", bufs=2) as xq_pool,
        ):
            ident_sb = constp.tile([128, 128], bf16)
            cs_sb = constp.tile([128, S], f32)       # cos on p0:64, sin 64:128
            tri_sb = constp.tile([128, 128], bf16)
            tri2_sb = constp.tile([128, 256], bf16)
            onesc_sb = constp.tile([128, 1], bf16)
            onesr_sb = constp.tile([1, 128], bf16)
            sel4_sb = constp.tile([128, 1], bf16)
            zeros_sb = constp.tile([128, 128], bf16)
            # ident first on the sync (HWDGE) ring: warm-up matmuls + the V
            # transposes need it
            nc.sync.dma_start(ident_sb[:], ident)
            nc.gpsimd.dma_start(onesc_sb[:], onesc)
            nc.gpsimd.dma_start(onesr_sb[:], onesr)
            nc.gpsimd.dma_start(sel4_sb[:], sel4)
            nc.gpsimd.dma_start(cs_sb[:], cs)
            nc.gpsimd.dma_start(tri_sb[:], tri)
            nc.gpsimd.dma_start(tri2_sb[:], tri2)
            nc.vector.memset(zeros_sb[:], 0.0)
            # prewarm the Exp activation table during the DMA-bound startup
            warm = constp.tile([1, 1], f32)
            nc.scalar.activation(warm[:], onesc_sb[0:1, 0:1],
                                 mybir.ActivationFunctionType.Exp, scale=1.0)

            # residents across phases (wt is chunk-major: [dk, m] so one DMA
            # per x-chunk brings all 6 m-tiles' weights for those k-tiles)
            wt_sb = resid.tile([128, DK, NM, 128], bf16)   # 48KB/part
            qt_sb = resid.tile([128, QH, S], bf16)         # roped Q^T
            kt_sb = resid.tile([128, S], bf16)             # roped K^T
            v_sb = resid.tile([128, ST, HD], bf16)         # natural V
            ot_sb = resid.tile([128, QH, S], bf16)         # normalized O^T

            # ---------------- Phase 1: QKV projections + RoPE. Chunk-major:
            # per x-chunk, all 6 m-tiles accumulate in 6 PSUM banks, so the
            # DMA-gated first quarter still gets 24 matmuls per chunk.
            with (
                tc.tile_pool(name="p1tmp", bufs=2) as p1tmp,
                tc.tile_pool(name="p1psum", bufs=1, space="PSUM") as p1psum,
                tc.tile_pool(name="tpsum", bufs=2, space="PSUM") as tpsum,
            ):
                # HAM warm-up: ~32 back-to-back matmuls on ident flip the PE
                # clock gate to 8/8 while the first x chunks stream in
                wu = tpsum.tile([128, 128], f32, tag="tp", name="wu")
                for _ in range(32):
                    nc.tensor.matmul(wu[:], lhsT=ident_sb[:], rhs=ident_sb[:],
                                     start=True, stop=True)

                for q in range(4):
                    qsl = slice(q * SG, (q + 1) * SG)
                    xtd_q = xq_pool.tile([128, DK, SG], bf16, tag="xtd_q",
                                         name="xtd_q")
                    # exact FIFO order on the sync ring: for q0, each chunk's
                    # weights (all 6 m-tiles) land just before its x slice,
                    # so the chunk-major matmuls track DMA arrival
                    if q == 0:
                        nchunk, w = 8, 4
                        for c in range(nchunk):
                            ksl = slice(c * w, (c + 1) * w)
                            nc.sync.dma_start(wt_sb[:, ksl], wt[:, ksl])
                            nc.sync.dma_start(xtd_q[:, ksl, :],
                                              xtd[:, q, ksl, :])
                    else:
                        nchunk, w = 4, 8
                        for c in range(nchunk):
                            ksl = slice(c * w, (c + 1) * w)
                            nc.sync.dma_start(xtd_q[:, ksl, :],
                                              xtd[:, q, ksl, :])

                    qp = [p1psum.tile([128, SG], f32, tag=f"qp{m}",
                                      name=f"qp{m}") for m in range(NM)]
                    for c in range(nchunk):
                        for kt in range(c * w, (c + 1) * w):
                            for m in range(NM):
                                nc.tensor.matmul(qp[m][:],
                                                 lhsT=wt_sb[:, kt, m],
                                                 rhs=xtd_q[:, kt, :],
                                                 start=(kt == 0),
                                                 stop=(kt == DK - 1),
                                                 skip_group_check=True)

                    # quarter-end evictions: V first (fast PSUM release),
                    # then the rope chains for K and Q
                    vt = p1tmp.tile([128, SG], bf16, name="vt")
                    nc.vector.tensor_copy(vt[:], qp[0][:])
                    for j in range(4):
                        stile = q * 4 + j
                        tp = tpsum.tile([128, 128], bf16, name="tp", tag="tp")
                        nc.tensor.transpose(
                            tp[:], vt[:, j * 128:(j + 1) * 128], ident_sb[:])
                        nc.vector.tensor_copy(v_sb[:, stile, :], tp[:])
                    for m in range(1, NM):
                        # rope in transposed layout: pairs are (p, p+64)
                        qa, qb = qp[m][0:64, :], qp[m][64:128, :]
                        c_q = cs_sb[0:64, qsl]
                        s_q = cs_sb[64:128, qsl]
                        ta = p1tmp.tile([64, SG], f32, name="ta")
                        tb = p1tmp.tile([64, SG], f32, name="tb")
                        tc1 = p1tmp.tile([64, SG], f32, name="tc1")
                        tc2 = p1tmp.tile([64, SG], f32, name="tc1")
                        if m == 1:
                            dst_a = kt_sb[0:64, qsl]
                            dst_b = kt_sb[64:128, qsl]
                        else:
                            dst_a = qt_sb[0:64, m - 2, qsl]
                            dst_b = qt_sb[64:128, m - 2, qsl]
                        nc.vector.tensor_tensor(ta[:], qa, s_q, mul)
                        nc.vector.tensor_tensor(tb[:], qb, s_q, mul)
                        nc.vector.tensor_tensor(tc1[:], qa, c_q, mul)
                        nc.vector.tensor_tensor(dst_a, tc1[:], tb[:],
                                                mybir.AluOpType.subtract)
                        nc.vector.tensor_tensor(tc2[:], qb, c_q, mul)
                        nc.vector.tensor_tensor(dst_b, tc2[:], ta[:], add)

                # wo streams into the x double-buffer slot freed by q2
                # (same tag, same 32KB/partition); per-head DMAs so phase 3
                # can start on head 0 before the tail heads land
                wo_sb = xq_pool.tile([128, QH, DK, 128], bf16, tag="xtd_q",
                                     name="wo_sb")
                for h in range(QH):
                    nc.gpsimd.dma_start(wo_sb[:, h], wo[:, h])

            # ---------------- Phase 2 + 3 interleaved: attention is
            # group-outer / head-inner; after each group's 4 heads, that
            # group's phase-3 chunk is emitted so the scheduler can fill
            # exp-wait stalls with out-projection matmuls.
            with (
                tc.tile_pool(name="p2tmp", bufs=3) as p2tmp,
                tc.tile_pool(name="p2lb", bufs=1) as p2lb,
                tc.tile_pool(name="stpsum", bufs=1, space="PSUM") as stpsum,
                tc.tile_pool(name="otpsum", bufs=1, space="PSUM") as otpsum,
                tc.tile_pool(name="p3psum", bufs=1, space="PSUM") as p3psum,
                tc.tile_pool(name="p3out", bufs=3) as p3out,
            ):
                def attention(g, h):
                    gsl = slice(g * SG, (g + 1) * SG)
                    nk = 4 * g + 4
                    npair = nk // 2
                    ot_ps = otpsum.tile([128, SG], f32, name="ot_ps",
                                        tag="ot_ps", bufs=1)
                    l_ps = otpsum.tile([128, SG], f32, name="l_ps",
                                       tag="l_ps", bufs=1)
                    # zero-weight N=1 matmul: clears the l bank's
                    # has_written bits on all 128 partitions and writes 0.0
                    # to column 0, so every later l matmul can accumulate
                    # with start=False (the 4 col groups must not each
                    # start=True: that would wipe each other's bits)
                    nc.tensor.matmul(l_ps[:, 0:1], lhsT=zeros_sb[:],
                                     rhs=onesc_sb[:], start=True, stop=False,
                                     skip_group_check=True)
                    pair_tiles = {}

                    def poff(p):
                        return max(0, (2 * p - 4 * g)) * 128

                    def do_pair(p):
                        off = poff(p)
                        stp = stpsum.tile([128, 2, SG], f32, name="st_ps",
                                          tag="st_ps", bufs=2)
                        for i in range(2):
                            j = 2 * p + i
                            nc.tensor.matmul(
                                stp[:, i, off:],
                                lhsT=kt_sb[:, j * 128:(j + 1) * 128],
                                rhs=qt_sb[:, h, g * SG + off:(g + 1) * SG],
                                start=True, stop=True)
                        pair_tiles[p] = stp

                    for p in range(min(2, npair)):
                        do_pair(p)
                    put_prev = None
                    for p in range(npair):
                        off = poff(p)
                        stp = pair_tiles.pop(p)
                        put = p2tmp.tile([128, 2, SG], bf16, name="put",
                                         bufs=3)
                        nc.scalar.activation(
                            put[:, :, off:], stp[:, :, off:],
                            mybir.ActivationFunctionType.Exp, scale=SCALE)
                        if 2 * p >= 4 * g:   # diagonal pair
                            nc.vector.tensor_tensor(
                                put[:, 0, off:off + 128],
                                put[:, 0, off:off + 128],
                                tri_sb[:], mul)
                            nc.vector.tensor_tensor(
                                put[:, 1, off:off + 256],
                                put[:, 1, off:off + 256],
                                tri2_sb[:], mul)
                        for i in range(2):
                            j = 2 * p + i
                            offj = max(0, (j - 4 * g)) * 128
                            nc.tensor.matmul(ot_ps[:, offj:],
                                             lhsT=v_sb[:, j, :],
                                             rhs=put[:, i, offj:],
                                             start=(j == 0),
                                             stop=(j == nk - 1),
                                             skip_group_check=True)
                        # softmax denominators: M=1 row-sum matmuls.
                        # g==0: plain serial into partition 0. g>=1: 4
                        # consecutive tiles col-tiled into PE column groups
                        # 0..3 -> they run concurrently on the array and the
                        # partials land on partitions 0/32/64/96.
                        if g == 0:
                            for i in range(2):
                                j = 2 * p + i
                                offj = j * 128
                                nc.tensor.matmul(
                                    l_ps[0:1, offj:], lhsT=onesc_sb[:],
                                    rhs=put[:, i, offj:], start=False,
                                    stop=(j == nk - 1),
                                    skip_group_check=True)
                        elif p % 2 == 1:
                            for c in range(4):
                                j = 4 * (p // 2) + c
                                offj = max(0, (j - 4 * g)) * 128
                                src = put_prev if c < 2 else put
                                nc.tensor.matmul(
                                    l_ps[32 * c:32 * c + 1, offj:],
                                    lhsT=onesc_sb[:],
                                    rhs=src[:, c % 2, offj:], start=False,
                                    stop=(j == nk - 1),
                                    skip_group_check=True,
                                    tile_position=(0, 32 * c))
                        put_prev = put
                        if p + 2 < npair:
                            do_pair(p + 2)

                    # finalize: sum the 4 l partials, reciprocal, bf16
                    # broadcast back into the (re-cleared) l bank, normalize
                    if g == 0:
                        lsrc = l_ps[0:1, :]
                    else:
                        # sum the 4 partial rows with one selector matmul
                        # (ones at partitions 0/32/64/96): DVE can't add two
                        # PSUM operands and both-SBUF adds need equal base
                        # partitions, but the PE contracts partitions for
                        # free. One bank copy to SBUF (DVE cost is
                        # per-column, extra partitions are free) + one N=512
                        # matmul back into the re-cleared l bank.
                        l4 = p2lb.tile([128, SG], bf16, name="l4")
                        nc.vector.tensor_copy(l4[:], l_ps[:])
                        nc.tensor.matmul(l_ps[0:1, :], lhsT=sel4_sb[:],
                                         rhs=l4[:], start=True, stop=True,
                                         skip_group_check=True)
                        lsrc = l_ps[0:1, :]
                    linv = p2lb.tile([1, SG], f32, name="linv")
                    nc.vector.reciprocal_approx_fast(linv[:], lsrc)
                    linv_b = p2lb.tile([1, SG], bf16, name="linv_b")
                    nc.vector.tensor_copy(linv_b[:], linv[:])
                    nc.tensor.matmul(l_ps[:, :], lhsT=onesr_sb[:],
                                     rhs=linv_b[:], start=True, stop=True,
                                     skip_group_check=True)
                    lb_sb = p2lb.tile([128, SG], f32, name="lb_sb")
                    nc.vector.tensor_copy(lb_sb[:], l_ps[:])
                    nc.vector.tensor_tensor(ot_sb[:, h, gsl], ot_ps[:],
                                            lb_sb[:], mul)

                def p3_block(gsrc):
                    for dt in range(DK):
                        y_ps = p3psum.tile([128, SG], f32, name="y_ps",
                                           tag="y_ps", bufs=2)
                        for h in range(QH):
                            nc.tensor.matmul(
                                y_ps[:], lhsT=wo_sb[:, h, dt],
                                rhs=ot_sb[:, h, gsrc * SG:(gsrc + 1) * SG],
                                start=(h == 0), stop=(h == QH - 1),
                                skip_group_check=True)
                        y_sb = p3out.tile([128, SG], bf16, name="y_sb")
                        nc.vector.tensor_copy(y_sb[:], y_ps[:])
                        dst = yt[dt * 128:(dt + 1) * 128,
                                 gsrc * SG:(gsrc + 1) * SG]
                        if (dt + gsrc) % 2 == 0:
                            nc.sync.dma_start(dst, y_sb[:])
                        else:
                            nc.gpsimd.dma_start(dst, y_sb[:])

                for g in range(4):
                    for h in range(QH):
                        attention(g, h)
                    if g > 0:
                        p3_block(g - 1)
                p3_block(3)
    nc.compile()
    return nc


def _host_prep(inputs):
    x = np.asarray(inputs["x"], dtype=np.float32).reshape(S, D)
    wq = np.asarray(inputs["wq"], dtype=np.float32)
    wk = np.asarray(inputs["wk"], dtype=np.float32)
    wv = np.asarray(inputs["wv"], dtype=np.float32)
    wo_full = np.asarray(inputs["wo"], dtype=np.float32)
    cos = np.asarray(inputs["freqs_cos"], dtype=np.float32)
    sin = np.asarray(inputs["freqs_sin"], dtype=np.float32)
    mask = np.asarray(inputs["mask"], dtype=np.float32)

    # xtd[p, q, kt, s] = x[512*q + s, 128*kt + p]
    xtd = np.ascontiguousarray(
        x.reshape(4, SG, DK, 128).transpose(3, 0, 2, 1)).astype(
            ml_dtypes.bfloat16)

    # de-interleave RoPE pairs within each head: evens then odds
    perm = np.concatenate([np.arange(0, HD, 2), np.arange(1, HD, 2)])

    # cos on partitions 0:64, sin on 64:128; [128, S]
    cs = np.ascontiguousarray(np.concatenate([cos.T, sin.T], axis=0))

    # causal keep-mask for the diagonal 128x128 block: tri[k, q] = keep
    tri_t = np.ascontiguousarray(
        (mask[0:128, 0:128].T == 0.0)).astype(ml_dtypes.bfloat16)

    def mtile(w):  # [D, 768] -> [128, DK, NM, 128] (chunk-major)
        return np.ascontiguousarray(
            w.reshape(DK, 128, NM, 128).transpose(1, 0, 2, 3))

    sel4 = np.zeros((128, 1), dtype=ml_dtypes.bfloat16)
    sel4[0::32, 0] = 1.0

    in_maps = []
    for c in range(NCORES):
        wq_c = wq[:, 512 * c:512 * (c + 1)].reshape(D, QH, HD)[:, :, perm]
        wq_c = wq_c.reshape(D, QH * HD)
        wk_c = wk[:, 128 * c:128 * (c + 1)][:, perm]
        wv_c = wv[:, 128 * c:128 * (c + 1)]
        W = np.concatenate([wv_c, wk_c, wq_c], axis=1)  # [D, 768]
        wo_c = wo_full[512 * c:512 * (c + 1), :]        # [512, D]
        wo_t = np.ascontiguousarray(
            wo_c.reshape(QH, 128, DK, 128).transpose(1, 0, 2, 3))
        in_maps.append({
            "xtd": xtd,
            "wt": mtile(W).astype(ml_dtypes.bfloat16),
            "wo": wo_t.astype(ml_dtypes.bfloat16),
            "cs": cs,
            "tri": tri_t,
            "tri2": np.ascontiguousarray(np.concatenate(
                [np.zeros((128, 128), np.float32), tri_t.astype(np.float32)],
                axis=1)).astype(ml_dtypes.bfloat16),
            "onesc": np.ones((128, 1), dtype=ml_dtypes.bfloat16),
            "onesr": np.ones((1, 128), dtype=ml_dtypes.bfloat16),
            "sel4": sel4,
            "ident": np.eye(128, dtype=ml_dtypes.bfloat16),
        })
    return in_maps


def _run(inputs, trace=False, tmpdir=None):
    from concourse import bass_utils
    if "nc" not in _CACHE:
        _CACHE["nc"] = _build()
    nc = _CACHE["nc"]
    in_maps = _host_prep(inputs)
    res = bass_utils.run_bass_kernel_spmd(
        nc, in_maps, core_ids=list(range(NCORES)), trace=trace, tmpdir=tmpdir)
    y = np.zeros((S, D), dtype=np.float32)
    for c in range(NCORES):
        y += res.results[c]["yt"].astype(np.float32).T
    return y.reshape(1, S, D), res


def kernel(**inputs):
    y, _ = _run(inputs, trace=False)
    return y


# revision 19
# speedup vs baseline: 1.0779x; 1.0779x over previous
"""GQA attention layer (B=1, S=2048, D=4096, H=32, KV=8, HD=128) on 8 TRN2
NeuronCores. Tensor-parallel over the 8 KV-head groups; NO collective: each
core computes a partial y = O_local @ wo_rows_local over the full output dim
and the host sums the 8 partials during unshard.

All matmuls in bf16 (fp32 PSUM accumulation). Phase 1 is weight-stationary,
chunk-major (6 live PSUM accumulators, one per m-tile) so compute tracks the
x-chunk DMA arrival during the startup quarter; qT/kT are produced directly
(RoPE applied in transposed layout at quarter-end eviction). The sync DMA
ring carries ident -> wt-m0/x-q0 interleaved -> wt-m1..5 -> x-q1.. in exact
FIFO order so every tensor lands just-in-time; warm-up matmuls on ident flip
the PE HAM clock gate to 8/8 before real work arrives.

Attention runs transposed (scores^T = kT-tiles as lhsT against qT); score
tiles are processed in PAIRS sharing one wide exp ACTIVATE. Softmax sums run
as M=1 matmuls col-tiled 4-way into separate PE column groups (concurrent on
the array, ~4x cheaper than serial row-sum matmuls); a zero-weight N=1
matmul pre-clears the PSUM bank's has_written bits so the col groups can all
accumulate with start=False. The 1/l broadcast matmul is bf16.

Phase 2 is group-outer/head-inner and each group's phase-3 out-projection
chunk (32 dt x 4 head matmuls into a [128,512] PSUM tile) is emitted right
after the group finishes, so the Tile scheduler hoists phase-3 matmuls into
phase-2's exp-wait stalls. wo streams into the SBUF slot freed by the x
double-buffer (same tag) mid-phase-1. Output is written per (dt, group)
chunk, alternating sync/gpsimd DMA queues.
"""
import numpy as np
import ml_dtypes

S = 2048
D = 4096
HD = 128
QH = 4            # q heads per core
NCORES = 8
ST = S // 128     # 16 s-tiles
DK = D // 128     # 32 contraction tiles
G = 4             # q groups per head
SG = S // G       # 512 q positions per group
NM = 6            # phase-1 m-tiles per core: v, k, q0..q3
SCALE = 1.0 / np.sqrt(128.0)

_CACHE = {}


def _build():
    import concourse.mybir as mybir
    import concourse.tile as tile
    from concourse import bacc

    f32 = mybir.dt.float32
    bf16 = mybir.dt.bfloat16
    mul = mybir.AluOpType.mult
    add = mybir.AluOpType.add
    nc = bacc.Bacc("TRN2", target_bir_lowering=False, debug=False,
                   num_devices=NCORES)

    xtd = nc.dram_tensor("xtd", [128, 4, DK, SG], bf16,
                         kind="ExternalInput").ap()
    wt = nc.dram_tensor("wt", [128, DK, NM, 128], bf16,
                        kind="ExternalInput").ap()
    wo = nc.dram_tensor("wo", [128, QH, DK, 128], bf16,
                        kind="ExternalInput").ap()
    cs = nc.dram_tensor("cs", [128, S], f32, kind="ExternalInput").ap()
    tri = nc.dram_tensor("tri", [128, 128], bf16, kind="ExternalInput").ap()
    tri2 = nc.dram_tensor("tri2", [128, 256], bf16, kind="ExternalInput").ap()
    onesc = nc.dram_tensor("onesc", [128, 1], bf16, kind="ExternalInput").ap()
    onesr = nc.dram_tensor("onesr", [1, 128], bf16, kind="ExternalInput").ap()
    sel4 = nc.dram_tensor("sel4", [128, 1], bf16, kind="ExternalInput").ap()
    ident = nc.dram_tensor("ident", [128, 128], bf16,
                           kind="ExternalInput").ap()
    yt = nc.dram_tensor("yt", [D, S], bf16, kind="ExternalOutput").ap()

    with tile.TileContext(nc) as tc:
        with (
            tc.tile_pool(name="const", bufs=1) as constp,
            tc.tile_pool(name="resid", bufs=1) as resid,
            tc.tile_pool(name="xq_pool", bufs=2) as xq_pool,
        ):
            ident_sb = constp.tile([128, 128], bf16)
            cs_sb = constp.tile([128, S], f32)       # cos on p0:64, sin 64:128
            tri_sb = constp.tile([128, 128], bf16)
            tri2_sb = constp.tile([128, 256], bf16)
            onesc_sb = constp.tile([128, 1], bf16)
            onesr_sb = constp.tile([1, 128], bf16)
            sel4_sb = constp.tile([128, 1], bf16)
            zeros_sb = constp.tile([128, 128], bf16)
            # ident first on the sync (HWDGE) ring: warm-up matmuls + the V
            # transposes need it
            nc.sync.dma_start(ident_sb[:], ident)
            nc.gpsimd.dma_start(onesc_sb[:], onesc)
            nc.gpsimd.dma_start(onesr_sb[:], onesr)
            nc.gpsimd.dma_start(sel4_sb[:], sel4)
            nc.gpsimd.dma_start(cs_sb[:], cs)
            nc.gpsimd.dma_start(tri_sb[:], tri)
            nc.gpsimd.dma_start(tri2_sb[:], tri2)
            nc.vector.memset(zeros_sb[:], 0.0)
            # prewarm the Exp activation table during the DMA-bound startup
            warm = constp.tile([1, 1], f32)
            nc.scalar.activation(warm[:], onesc_sb[0:1, 0:1],
                                 mybir.ActivationFunctionType.Exp, scale=1.0)

            # residents across phases (wt is chunk-major: [dk, m] so one DMA
            # per x-chunk brings all 6 m-tiles' weights for those k-tiles)
            wt_sb = resid.tile([128, DK, NM, 128], bf16)   # 48KB/part
            qt_sb = resid.tile([128, QH, S], bf16)         # roped Q^T
            kt_sb = resid.tile([128, S], bf16)             # roped K^T
            v_sb = resid.tile([128, ST, HD], bf16)         # natural V
            ot_sb = resid.tile([128, QH, S], bf16)         # normalized O^T

            # ---------------- Phase 1: QKV projections + RoPE. Chunk-major:
            # per x-chunk, all 6 m-tiles accumulate in 6 PSUM banks, so the
            # DMA-gated first quarter still gets 24 matmuls per chunk.
            with (
                tc.tile_pool(name="p1tmp", bufs=2) as p1tmp,
                tc.tile_pool(name="p1psum", bufs=1, space="PSUM") as p1psum,
                tc.tile_pool(name="tpsum", bufs=2, space="PSUM") as tpsum,
            ):
                # HAM warm-up: back-to-back matmuls on ident flip the PE
                # clock gate to 8/8 while the first x chunks stream in.
                # Batches are staggered (more emitted between the first
                # chunks' matmuls) so DMA-wait idle gaps don't re-throttle
                # the clock before real work is dense.
                def warmup(n):
                    wu = tpsum.tile([128, 128], f32, tag="tp", name="wu")
                    for _ in range(n):
                        nc.tensor.matmul(wu[:], lhsT=ident_sb[:],
                                         rhs=ident_sb[:],
                                         start=True, stop=True)
                warmup(24)

                def rope_evict(qpm, m, qsl):
                    # rope in transposed layout: pairs are (p, p+64)
                    qa, qb = qpm[0:64, :], qpm[64:128, :]
                    c_q = cs_sb[0:64, qsl]
                    s_q = cs_sb[64:128, qsl]
                    ta = p1tmp.tile([64, SG], f32, name="ta")
                    tb = p1tmp.tile([64, SG], f32, name="tb")
                    tc1 = p1tmp.tile([64, SG], f32, name="tc1")
                    tc2 = p1tmp.tile([64, SG], f32, name="tc1")
                    if m == 1:
                        dst_a = kt_sb[0:64, qsl]
                        dst_b = kt_sb[64:128, qsl]
                    else:
                        dst_a = qt_sb[0:64, m - 2, qsl]
                        dst_b = qt_sb[64:128, m - 2, qsl]
                    nc.vector.tensor_tensor(ta[:], qa, s_q, mul)
                    nc.vector.tensor_tensor(tb[:], qb, s_q, mul)
                    nc.vector.tensor_tensor(tc1[:], qa, c_q, mul)
                    nc.vector.tensor_tensor(dst_a, tc1[:], tb[:],
                                            mybir.AluOpType.subtract)
                    nc.vector.tensor_tensor(tc2[:], qb, c_q, mul)
                    nc.vector.tensor_tensor(dst_b, tc2[:], ta[:], add)

                def v_evict(qpm, q):
                    vt = p1tmp.tile([128, SG], bf16, name="vt")
                    nc.vector.tensor_copy(vt[:], qpm[:])
                    for j in range(4):
                        stile = q * 4 + j
                        tp = tpsum.tile([128, 128], bf16, name="tp", tag="tp")
                        nc.tensor.transpose(
                            tp[:], vt[:, j * 128:(j + 1) * 128], ident_sb[:])
                        nc.vector.tensor_copy(v_sb[:, stile, :], tp[:])

                for q in range(4):
                    qsl = slice(q * SG, (q + 1) * SG)
                    xtd_q = xq_pool.tile([128, DK, SG], bf16, tag="xtd_q",
                                         name="xtd_q")
                    # exact FIFO order on the sync ring: for q0, each chunk's
                    # weights (all 6 m-tiles) land just before its x slice,
                    # so the chunk-major matmuls track DMA arrival
                    if q == 0:
                        nchunk, w = 8, 4
                        for c in range(nchunk):
                            ksl = slice(c * w, (c + 1) * w)
                            nc.sync.dma_start(wt_sb[:, ksl], wt[:, ksl])
                            nc.sync.dma_start(xtd_q[:, ksl, :],
                                              xtd[:, q, ksl, :])
                    else:
                        nchunk, w = 4, 8
                        for c in range(nchunk):
                            ksl = slice(c * w, (c + 1) * w)
                            nc.sync.dma_start(xtd_q[:, ksl, :],
                                              xtd[:, q, ksl, :])

                    if q == 0:
                        # chunk-major: all 6 m-tiles accumulate per chunk so
                        # the DMA-gated startup quarter gets 24 matmuls per
                        # chunk; evictions bunch at quarter end but overlap
                        # q1's matmuls. Warm-up batches keep the PE clock
                        # from re-throttling during early chunk waits.
                        qp = [p1psum.tile([128, SG], f32, tag=f"qp{m}",
                                          name=f"qp{m}") for m in range(NM)]
                        for c in range(nchunk):
                            for kt in range(c * w, (c + 1) * w):
                                for m in range(NM):
                                    nc.tensor.matmul(qp[m][:],
                                                     lhsT=wt_sb[:, kt, m],
                                                     rhs=xtd_q[:, kt, :],
                                                     start=(kt == 0),
                                                     stop=(kt == DK - 1),
                                                     skip_group_check=True)
                            if c < 3:
                                warmup(8)
                        v_evict(qp[0], q)
                        for m in range(1, NM):
                            rope_evict(qp[m], m, qsl)
                    else:
                        # m-major: one m-tile pass at a time, each evicted
                        # immediately so the rope chains spread across the
                        # quarter instead of bunching before phase 2. V last
                        # on q3: its eviction is one fast copy, so phase 2
                        # isn't gated on a long rope chain.
                        m_order = [1, 2, 3, 4, 5, 0] if q == 3 else range(NM)
                        for m in m_order:
                            qpm = p1psum.tile([128, SG], f32, tag=f"qp{m}",
                                              name=f"qp{m}")
                            for kt in range(DK):
                                nc.tensor.matmul(qpm[:],
                                                 lhsT=wt_sb[:, kt, m],
                                                 rhs=xtd_q[:, kt, :],
                                                 start=(kt == 0),
                                                 stop=(kt == DK - 1),
                                                 skip_group_check=True)
                            if m == 0:
                                v_evict(qpm, q)
                            else:
                                rope_evict(qpm, m, qsl)

                # wo streams into the x double-buffer slot freed by q2
                # (same tag, same 32KB/partition); per-head DMAs so phase 3
                # can start on head 0 before the tail heads land
                wo_sb = xq_pool.tile([128, QH, DK, 128], bf16, tag="xtd_q",
                                     name="wo_sb")
                for h in range(QH):
                    nc.gpsimd.dma_start(wo_sb[:, h], wo[:, h])

            # ---------------- Phase 2 + 3 interleaved: attention is
            # group-outer / head-inner; after each group's 4 heads, that
            # group's phase-3 chunk is emitted so the scheduler can fill
            # exp-wait stalls with out-projection matmuls.
            with (
                tc.tile_pool(name="p2tmp", bufs=3) as p2tmp,
                tc.tile_pool(name="p2lb", bufs=1) as p2lb,
                tc.tile_pool(name="stpsum", bufs=1, space="PSUM") as stpsum,
                tc.tile_pool(name="otpsum", bufs=1, space="PSUM") as otpsum,
                tc.tile_pool(name="p3psum", bufs=1, space="PSUM") as p3psum,
                tc.tile_pool(name="p3out", bufs=3) as p3out,
            ):
                def attention(g, h):
                    gsl = slice(g * SG, (g + 1) * SG)
                    nk = 4 * g + 4
                    npair = nk // 2
                    ot_ps = otpsum.tile([128, SG], f32, name="ot_ps",
                                        tag="ot_ps", bufs=1)
                    l_ps = otpsum.tile([128, SG], f32, name="l_ps",
                                       tag="l_ps", bufs=1)
                    # zero-weight N=1 matmul: clears the l bank's
                    # has_written bits on all 128 partitions and writes 0.0
                    # to column 0, so every later l matmul can accumulate
                    # with start=False (the 4 col groups must not each
                    # start=True: that would wipe each other's bits)
                    nc.tensor.matmul(l_ps[:, 0:1], lhsT=zeros_sb[:],
                                     rhs=onesc_sb[:], start=True, stop=False,
                                     skip_group_check=True)
                    pair_tiles = {}

                    def poff(p):
                        return max(0, (2 * p - 4 * g)) * 128

                    def do_pair(p):
                        off = poff(p)
                        stp = stpsum.tile([128, 2, SG], f32, name="st_ps",
                                          tag="st_ps", bufs=2)
                        for i in range(2):
                            j = 2 * p + i
                            nc.tensor.matmul(
                                stp[:, i, off:],
                                lhsT=kt_sb[:, j * 128:(j + 1) * 128],
                                rhs=qt_sb[:, h, g * SG + off:(g + 1) * SG],
                                start=True, stop=True)
                        pair_tiles[p] = stp

                    for p in range(min(2, npair)):
                        do_pair(p)
                    put_prev = None
                    for p in range(npair):
                        off = poff(p)
                        stp = pair_tiles.pop(p)
                        put = p2tmp.tile([128, 2, SG], bf16, name="put",
                                         bufs=3)
                        nc.scalar.activation(
                            put[:, :, off:], stp[:, :, off:],
                            mybir.ActivationFunctionType.Exp, scale=SCALE)
                        if 2 * p >= 4 * g:   # diagonal pair
                            nc.vector.tensor_tensor(
                                put[:, 0, off:off + 128],
                                put[:, 0, off:off + 128],
                                tri_sb[:], mul)
                            nc.vector.tensor_tensor(
                                put[:, 1, off:off + 256],
                                put[:, 1, off:off + 256],
                                tri2_sb[:], mul)
                        for i in range(2):
                            j = 2 * p + i
                            offj = max(0, (j - 4 * g)) * 128
                            nc.tensor.matmul(ot_ps[:, offj:],
                                             lhsT=v_sb[:, j, :],
                                             rhs=put[:, i, offj:],
                                             start=(j == 0),
                                             stop=(j == nk - 1),
                                             skip_group_check=True)
                        # softmax denominators: M=1 row-sum matmuls.
                        # g==0: plain serial into partition 0. g>=1: 4
                        # consecutive tiles col-tiled into PE column groups
                        # 0..3 -> they run concurrently on the array and the
                        # partials land on partitions 0/32/64/96.
                        if g == 0:
                            for i in range(2):
                                j = 2 * p + i
                                offj = j * 128
                                nc.tensor.matmul(
                                    l_ps[0:1, offj:], lhsT=onesc_sb[:],
                                    rhs=put[:, i, offj:], start=False,
                                    stop=(j == nk - 1),
                                    skip_group_check=True)
                        elif p % 2 == 1:
                            for c in range(4):
                                j = 4 * (p // 2) + c
                                offj = max(0, (j - 4 * g)) * 128
                                src = put_prev if c < 2 else put
                                nc.tensor.matmul(
                                    l_ps[32 * c:32 * c + 1, offj:],
                                    lhsT=onesc_sb[:],
                                    rhs=src[:, c % 2, offj:], start=False,
                                    stop=(j == nk - 1),
                                    skip_group_check=True,
                                    tile_position=(0, 32 * c))
                        put_prev = put
                        if p + 2 < npair:
                            do_pair(p + 2)

                    # finalize: sum the 4 l partials, reciprocal, bf16
                    # broadcast back into the (re-cleared) l bank, normalize
                    if g == 0:
                        lsrc = l_ps[0:1, :]
                    else:
                        # sum the 4 partial rows with one selector matmul
                        # (ones at partitions 0/32/64/96): DVE can't add two
                        # PSUM operands and both-SBUF adds need equal base
                        # partitions, but the PE contracts partitions for
                        # free. One bank copy to SBUF (DVE cost is
                        # per-column, extra partitions are free) + one N=512
                        # matmul back into the re-cleared l bank.
                        l4 = p2lb.tile([128, SG], bf16, name="l4")
                        nc.vector.tensor_copy(l4[:], l_ps[:])
                        nc.tensor.matmul(l_ps[0:1, :], lhsT=sel4_sb[:],
                                         rhs=l4[:], start=True, stop=True,
                                         skip_group_check=True)
                        lsrc = l_ps[0:1, :]
                    linv = p2lb.tile([1, SG], f32, name="linv")
                    nc.vector.reciprocal_approx_fast(linv[:], lsrc)
                    linv_b = p2lb.tile([1, SG], bf16, name="linv_b")
                    nc.vector.tensor_copy(linv_b[:], linv[:])
                    nc.tensor.matmul(l_ps[:, :], lhsT=onesr_sb[:],
                                     rhs=linv_b[:], start=True, stop=True,
                                     skip_group_check=True)
                    lb_sb = p2lb.tile([128, SG], f32, name="lb_sb")
                    nc.vector.tensor_copy(lb_sb[:], l_ps[:])
                    nc.vector.tensor_tensor(ot_sb[:, h, gsl], ot_ps[:],
                                            lb_sb[:], mul)

                def p3_block(gsrc, last=False):
                    for dt in range(DK):
                        # the final block has no phase-2 work to hide the
                        # PSUM-evict latency behind, so rotate through the
                        # (now idle) attention banks as extra slots
                        if last and dt % 3 == 2:
                            y_ps = stpsum.tile([128, SG], f32, name="y_ps2",
                                               tag="st_ps", bufs=2)
                        else:
                            y_ps = p3psum.tile([128, SG], f32, name="y_ps",
                                               tag="y_ps", bufs=2)
                        for h in range(QH):
                            nc.tensor.matmul(
                                y_ps[:], lhsT=wo_sb[:, h, dt],
                                rhs=ot_sb[:, h, gsrc * SG:(gsrc + 1) * SG],
                                start=(h == 0), stop=(h == QH - 1),
                                skip_group_check=True)
                        y_sb = p3out.tile([128, SG], bf16, name="y_sb")
                        nc.vector.tensor_copy(y_sb[:], y_ps[:])
                        dst = yt[dt * 128:(dt + 1) * 128,
                                 gsrc * SG:(gsrc + 1) * SG]
                        if (dt + gsrc) % 2 == 0:
                            nc.sync.dma_start(dst, y_sb[:])
                        else:
                            nc.gpsimd.dma_start(dst, y_sb[:])

                for g in range(4):
                    for h in range(QH):
                        attention(g, h)
                    if g > 0:
                        p3_block(g - 1)
                p3_block(3, last=True)
    nc.compile()
    return nc


def _host_prep(inputs):
    x = np.asarray(inputs["x"], dtype=np.float32).reshape(S, D)
    wq = np.asarray(inputs["wq"], dtype=np.float32)
    wk = np.asarray(inputs["wk"], dtype=np.float32)
    wv = np.asarray(inputs["wv"], dtype=np.float32)
    wo_full = np.asarray(inputs["wo"], dtype=np.float32)
    cos = np.asarray(inputs["freqs_cos"], dtype=np.float32)
    sin = np.asarray(inputs["freqs_sin"], dtype=np.float32)
    mask = np.asarray(inputs["mask"], dtype=np.float32)

    # xtd[p, q, kt, s] = x[512*q + s, 128*kt + p]
    xtd = np.ascontiguousarray(
        x.reshape(4, SG, DK, 128).transpose(3, 0, 2, 1)).astype(
            ml_dtypes.bfloat16)

    # de-interleave RoPE pairs within each head: evens then odds
    perm = np.concatenate([np.arange(0, HD, 2), np.arange(1, HD, 2)])

    # cos on partitions 0:64, sin on 64:128; [128, S]
    cs = np.ascontiguousarray(np.concatenate([cos.T, sin.T], axis=0))

    # causal keep-mask for the diagonal 128x128 block: tri[k, q] = keep
    tri_t = np.ascontiguousarray(
        (mask[0:128, 0:128].T == 0.0)).astype(ml_dtypes.bfloat16)

    def mtile(w):  # [D, 768] -> [128, DK, NM, 128] (chunk-major)
        return np.ascontiguousarray(
            w.reshape(DK, 128, NM, 128).transpose(1, 0, 2, 3))

    sel4 = np.zeros((128, 1), dtype=ml_dtypes.bfloat16)
    sel4[0::32, 0] = 1.0

    in_maps = []
    for c in range(NCORES):
        wq_c = wq[:, 512 * c:512 * (c + 1)].reshape(D, QH, HD)[:, :, perm]
        wq_c = wq_c.reshape(D, QH * HD)
        wk_c = wk[:, 128 * c:128 * (c + 1)][:, perm]
        wv_c = wv[:, 128 * c:128 * (c + 1)]
        W = np.concatenate([wv_c, wk_c, wq_c], axis=1)  # [D, 768]
        wo_c = wo_full[512 * c:512 * (c + 1), :]        # [512, D]
        wo_t = np.ascontiguousarray(
            wo_c.reshape(QH, 128, DK, 128).transpose(1, 0, 2, 3))
        in_maps.append({
            "xtd": xtd,
            "wt": mtile(W).astype(ml_dtypes.bfloat16),
            "wo": wo_t.astype(ml_dtypes.bfloat16),
            "cs": cs,
            "tri": tri_t,
            "tri2": np.ascontiguousarray(np.concatenate(
                [np.zeros((128, 128), np.float32), tri_t.astype(np.float32)],
                axis=1)).astype(ml_dtypes.bfloat16),
            "onesc": np.ones((128, 1), dtype=ml_dtypes.bfloat16),
            "onesr": np.ones((1, 128), dtype=ml_dtypes.bfloat16),
            "sel4": sel4,
            "ident": np.eye(128, dtype=ml_dtypes.bfloat16),
        })
    return in_maps


def _run(inputs, trace=False, tmpdir=None):
    from concourse import bass_utils
    if "nc" not in _CACHE:
        _CACHE["nc"] = _build()
    nc = _CACHE["nc"]
    in_maps = _host_prep(inputs)
    res = bass_utils.run_bass_kernel_spmd(
        nc, in_maps, core_ids=list(range(NCORES)), trace=trace, tmpdir=tmpdir)
    y = np.zeros((S, D), dtype=np.float32)
    for c in range(NCORES):
        y += res.results[c]["yt"].astype(np.float32).T
    return y.reshape(1, S, D), res


def kernel(**inputs):
    y, _ = _run(inputs, trace=False)
    return y
